# revision 8
# baseline (speedup 1.0000x reference)
"""L2L optimizer step (small MLP + LSTM cell + linear head) on 8 TRN2 cores.

Data-parallel over the batch/parameter dim B=524288 (65536 rows/core).

Dataflow (transposed: feature-dim on partitions, batch on free dim):
  x_pre = W1' @ [g; p; mom; 1]          (PE, K=4, f32)   [64, F] PSUM
  rx    = relu(x_pre)                   (DVE)            -> stack[0:64] bf16
  stack = [rx; h0T]                     (h0T DMA'd straight into rows 64:128)
  gates = [Wf; W_hh]^T-chunks @ stack   (PE, K=128, bf16) 2 chunks [128, F]
          where Wf = W_ih @ W2 (the MLP second layer is folded into the
          gate matmul) and gate biases ride on the ACT ops.
  sigmoid via tanh: sig(x) = 0.5*(tanh(x/2)+1), so every transcendental is
  Tanh (single ACT table, no table reloads):
    TA = tanh(0.5*GA + 0.5*bgA)         chunk (f, i)
    TB = tanh(sB*GB + bB)               chunk (o, g), per-partition scale
    A  = (tf+1)*c0T      B = (ti+1)*tg   C = A + B   (= 2*c1)
    tc1 = tanh(0.5*C)    h1' = (to+1)*tc1 (= 2*h1)
  update = (0.5*W_out) @ h1' + b_out    (PE, sliding stationary window puts
          block j's update row in PSUM partition j%64; drained twice/core)
"""
import numpy as np

B = 524288
H = 64
NCORES = 8
R = B // NCORES          # 65536 rows per core
F = 512                  # batch tile (moving free dim)
NBLK = R // F            # 128 blocks per core
FEATS_GRP = 8            # blocks per feats DMA


def _build_program():
    import concourse.bass as bass
    import concourse.tile as tile
    import concourse.mybir as mybir
    import tile_patch
    tile_patch.apply()

    F32 = mybir.dt.float32
    BF16 = mybir.dt.bfloat16

    nc = bass.Bass("TRN2", target_bir_lowering=False, num_devices=NCORES)

    # per-core inputs (host pre-sharded / pre-laid-out)
    feats_d = nc.dram_tensor("feats3", [3, R], F32, kind="ExternalInput")
    h0T_d = nc.dram_tensor("h0T", [H, R], BF16, kind="ExternalInput")
    c0T_d = nc.dram_tensor("c0T", [H, R], BF16, kind="ExternalInput")
    S1_d = nc.dram_tensor("S1", [3, H], F32, kind="ExternalInput")
    b1_d = nc.dram_tensor("b1c", [H, 1], F32, kind="ExternalInput")
    SGA_d = nc.dram_tensor("SGA", [128, 128], BF16, kind="ExternalInput")
    SGB_d = nc.dram_tensor("SGB", [128, 128], BF16, kind="ExternalInput")
    Z_d = nc.dram_tensor("Z", [H, 127], BF16, kind="ExternalInput")
    bA_d = nc.dram_tensor("biasA", [128, 1], F32, kind="ExternalInput")
    bB_d = nc.dram_tensor("biasB", [128, 1], F32, kind="ExternalInput")
    sB_d = nc.dram_tensor("scaleB", [128, 1], F32, kind="ExternalInput")
    bout_d = nc.dram_tensor("b_out_v", [H, 1], F32, kind="ExternalInput")
    upd_d = nc.dram_tensor("update", [NBLK, F], F32, kind="ExternalOutput")

    TANH = mybir.ActivationFunctionType.Tanh
    ADD = mybir.AluOpType.add
    MULT = mybir.AluOpType.mult

    with tile.TileContext(nc) as tc:
        import contextlib
        with contextlib.ExitStack() as ctx:
            singles = ctx.enter_context(tc.tile_pool(name="singles", bufs=1))
            ftp = ctx.enter_context(tc.tile_pool(name="ftp", bufs=3))
            stackp = ctx.enter_context(tc.tile_pool(name="stackp", bufs=3))
            cp = ctx.enter_context(tc.tile_pool(name="cp", bufs=3))
            tp = ctx.enter_context(tc.tile_pool(name="tp", bufs=2))
            ewp = ctx.enter_context(tc.tile_pool(name="ewp", bufs=2))
            outp = ctx.enter_context(tc.tile_pool(name="outp", bufs=2))
            ps_x = ctx.enter_context(tc.tile_pool(name="ps_x", bufs=2, space="PSUM"))
            ps_g = ctx.enter_context(tc.tile_pool(name="ps_g", bufs=2, space="PSUM"))
            ps_b = ctx.enter_context(tc.tile_pool(name="ps_b", bufs=1, space="PSUM"))
            ps_u = ctx.enter_context(tc.tile_pool(name="ps_u", bufs=1, space="PSUM"))

            # --- persistent weights / consts ---
            S1 = singles.tile([3, H], F32)
            nc.sync.dma_start(out=S1, in_=S1_d[:])
            b1c = singles.tile([H, 1], F32)
            nc.sync.dma_start(out=b1c, in_=b1_d[:])
            SGA = singles.tile([128, 128], BF16)
            nc.sync.dma_start(out=SGA, in_=SGA_d[:])
            SGB = singles.tile([128, 128], BF16)
            nc.sync.dma_start(out=SGB, in_=SGB_d[:])
            Z = singles.tile([H, 127], BF16)
            nc.sync.dma_start(out=Z, in_=Z_d[:])
            bA = singles.tile([128, 1], F32)
            nc.sync.dma_start(out=bA, in_=bA_d[:])
            bB = singles.tile([128, 1], F32)
            nc.sync.dma_start(out=bB, in_=bB_d[:])
            sB = singles.tile([128, 1], F32)
            nc.sync.dma_start(out=sB, in_=sB_d[:])
            bout = singles.tile([H, 1], F32)
            nc.sync.dma_start(out=bout, in_=bout_d[:])


            for half in range(2):
                upd_ps = ps_u.tile([H, F], F32)
                for j in range(NBLK // 2):
                    b = half * (NBLK // 2) + j
                    grp = b // FEATS_GRP
                    if b % FEATS_GRP == 0:
                        ft_cur = ftp.tile([3, F * FEATS_GRP], F32, tag="ft")
                        nc.sync.dma_start(
                            out=ft_cur,
                            in_=feats_d[:, grp * F * FEATS_GRP:(grp + 1) * F * FEATS_GRP])
                    fts = ft_cur[:, (b % FEATS_GRP) * F:(b % FEATS_GRP + 1) * F]

                    stack = stackp.tile([128, F], BF16)
                    nc.sync.dma_start(out=stack[H:128, :],
                                      in_=h0T_d[:, b * F:(b + 1) * F])
                    cT = cp.tile([H, F], BF16)
                    nc.sync.dma_start(out=cT, in_=c0T_d[:, b * F:(b + 1) * F])

                    xpre = ps_x.tile([H, F], F32)
                    nc.tensor.matmul(xpre, S1, fts, start=True, stop=True)
                    # relu -> stack rows 0:64 (bf16)
                    nc.vector.tensor_scalar(out=stack[0:H, :], in0=xpre, scalar1=b1c,
                                            scalar2=0.0, op0=mybir.AluOpType.add,
                                            op1=mybir.AluOpType.max)

                    GA = ps_g.tile([128, F], F32, tag="GA")
                    nc.tensor.matmul(GA, SGA, stack, start=True, stop=True)
                    GB = ps_g.tile([128, F], F32, tag="GB")
                    nc.tensor.matmul(GB, SGB, stack, start=True, stop=True)

                    TA = tp.tile([128, F], BF16, tag="TA")
                    nc.scalar.activation(TA, GA, TANH, bias=bA, scale=0.5)
                    TB = tp.tile([128, F], BF16, tag="TB")
                    nc.scalar.activation(TB, GB, TANH, bias=bB, scale=sB)

                    # A = (tf+1)*c0T   [L]
                    A = ewp.tile([H, F], BF16, tag="A")
                    nc.vector.scalar_tensor_tensor(
                        out=A, in0=TA[0:H, :], scalar=1.0, in1=cT,
                        op0=ADD, op1=MULT)
                    # B = (ti+1)*tg -> psum upper half (f32)
                    Bp = ps_b.tile([128, F], F32)
                    nc.vector.scalar_tensor_tensor(
                        out=Bp[H:128, :], in0=TA[H:128, :], scalar=1.0,
                        in1=TB[H:128, :], op0=ADD, op1=MULT)
                    # C = A + B  (cross-half via PSUM operand)
                    C = ewp.tile([H, F], BF16, tag="C")
                    nc.vector.tensor_tensor(out=C, in0=A, in1=Bp[H:128, :], op=ADD)
                    # tc1 = tanh(0.5*C)
                    TC1 = ewp.tile([H, F], BF16, tag="TC1")
                    nc.scalar.activation(TC1, C, TANH, bias=0.0, scale=0.5)
                    # h1' = (to+1)*tc1
                    H1 = ewp.tile([H, F], BF16, tag="H1")
                    nc.vector.scalar_tensor_tensor(
                        out=H1, in0=TB[0:H, :], scalar=1.0, in1=TC1,
                        op0=ADD, op1=MULT)
                    # update row j%64 via sliding stationary window
                    nc.tensor.matmul(upd_ps, Z[:, 63 - (j % H):127 - (j % H)], H1,
                                     start=(j % H == 0), stop=(j % H == H - 1))

                # drain updates for this half: add b_out, copy psum->sbuf, DMA out
                upd_sb = outp.tile([H, F], F32)
                nc.vector.tensor_scalar_add(upd_sb, upd_ps, bout)
                nc.sync.dma_start(
                    out=upd_d[half * (NBLK // 2):(half + 1) * (NBLK // 2), :],
                    in_=upd_sb)
    return nc


_PROG = None


def kernel(**inputs):
    global _PROG
    import ml_dtypes
    from concourse.bass_utils import run_bass_kernel_spmd

    g = np.asarray(inputs["gradients"], np.float32).reshape(B)
    p = np.asarray(inputs["parameters"], np.float32).reshape(B)
    mom = np.asarray(inputs["momentum"], np.float32).reshape(B)
    h0 = np.asarray(inputs["h0"], np.float32)
    c0 = np.asarray(inputs["c0"], np.float32)
    W1 = np.asarray(inputs["W1"], np.float32)
    b1 = np.asarray(inputs["b1"], np.float32)
    W2 = np.asarray(inputs["W2"], np.float32)
    b2 = np.asarray(inputs["b2"], np.float32)
    W_ih = np.asarray(inputs["W_ih"], np.float32)
    W_hh = np.asarray(inputs["W_hh"], np.float32)
    b_ih = np.asarray(inputs["b_ih"], np.float32)
    b_hh = np.asarray(inputs["b_hh"], np.float32)
    W_out = np.asarray(inputs["W_out"], np.float32)
    b_out = np.asarray(inputs["b_out"], np.float32)

    bf16 = ml_dtypes.bfloat16

    # ---- host-side weight prep ----
    S1 = np.stack([W1[:, 0] + W1[:, 2], W1[:, 1], 0.9 * W1[:, 2]]) \
        .astype(np.float32)                                   # [3, 64]
    b1c = b1.reshape(H, 1).astype(np.float32)
    Wf = W_ih @ W2                                            # [256, 64]
    bg = b_ih + b_hh + W_ih @ b2                              # [256]
    iA = list(range(64, 128)) + list(range(0, 64))            # (f, i)
    iB = list(range(192, 256)) + list(range(128, 192))        # (o, g)

    def chunk(idx):
        return np.concatenate([Wf[idx, :].T, W_hh[idx, :].T], axis=0)  # [128,128]

    SGA = chunk(iA).astype(bf16)
    SGB = chunk(iB).astype(bf16)
    biasA = (0.5 * bg[iA]).reshape(128, 1).astype(np.float32)
    biasB = np.concatenate([0.5 * bg[iB[:64]], bg[iB[64:]]]) \
        .reshape(128, 1).astype(np.float32)
    scaleB = np.concatenate([np.full(64, 0.5), np.full(64, 1.0)]) \
        .reshape(128, 1).astype(np.float32)
    Z = np.zeros((H, 127), np.float32)
    Z[:, 63] = 0.5 * W_out[0]
    Z = Z.astype(bf16)
    bout_v = np.full((H, 1), b_out[0], np.float32)

    feats3 = np.stack([g, p, mom])                            # [3, B] f32
    h0T = np.ascontiguousarray(h0.T).astype(bf16)             # [64, B]
    c0T = np.ascontiguousarray(c0.T).astype(bf16)

    if _PROG is None:
        _PROG = _build_program()
    nc = _PROG

    in_maps = []
    for c in range(NCORES):
        sl = slice(c * R, (c + 1) * R)
        in_maps.append({
            "feats3": np.ascontiguousarray(feats3[:, sl]),
            "h0T": np.ascontiguousarray(h0T[:, sl]),
            "c0T": np.ascontiguousarray(c0T[:, sl]),
            "S1": S1, "b1c": b1c, "SGA": SGA, "SGB": SGB, "Z": Z,
            "biasA": biasA, "biasB": biasB, "scaleB": scaleB,
            "b_out_v": bout_v,
        })
    import os
    trace = os.environ.get("KERNEL_TRACE", "0") == "1"
    res = run_bass_kernel_spmd(nc, in_maps, list(range(NCORES)), trace=trace)
    global LAST_RESULT
    LAST_RESULT = res
    upd = np.concatenate([res.results[c]["update"].reshape(R)
                          for c in range(NCORES)])
    return upd.reshape(B, 1).astype(np.float32)


LAST_RESULT = None


def bench(in_maps=None, iters=20, **inputs):
    """Amortized-dispatch device timing: run the NEFF `iters` times with
    device-resident inputs, return sec/iter."""
    import time
    import jax
    import numpy as np
    from jax.sharding import Mesh, PartitionSpec, NamedSharding
    from jax.experimental.shard_map import shard_map
    import concourse.mybir as mybir
    from concourse import bass2jax
    from concourse.bass2jax import _bass_exec_p, install_neuronx_cc_hook

    global _PROG
    if _PROG is None:
        _PROG = _build_program()
    nc = _PROG
    install_neuronx_cc_hook()

    if in_maps is None:
        in_maps = _make_in_maps(**inputs)

    partition_name = nc.partition_id_tensor.name if nc.partition_id_tensor else None
    in_names, out_names, out_avals, zero_outs = [], [], [], []
    for alloc in nc.m.functions[0].allocations:
        if not isinstance(alloc, mybir.MemoryLocationSet):
            continue
        name = alloc.memorylocations[0].name
        if alloc.kind == "ExternalInput":
            if name != partition_name:
                in_names.append(name)
        elif alloc.kind == "ExternalOutput":
            out_names.append(name)
            shape = tuple(alloc.tensor_shape)
            dtype = mybir.dt.np(alloc.dtype)
            out_avals.append(jax.core.ShapedArray(shape, dtype))
            zero_outs.append(np.zeros(shape, dtype))
    n_params = len(in_names)
    n_outs = len(out_avals)
    in_names_all = list(in_names) + list(out_names)
    if partition_name is not None:
        in_names_all.append(partition_name)
    donate = tuple(range(n_params, n_params + n_outs))

    def _body(*args):
        operands = list(args)
        if partition_name is not None:
            operands.append(bass2jax.partition_id_tensor())
        return tuple(_bass_exec_p.bind(
            *operands, out_avals=tuple(out_avals), in_names=tuple(in_names_all),
            out_names=tuple(out_names), lowering_input_output_aliases=(),
            sim_require_finite=True, sim_require_nnan=True, nc=nc))

    devices = jax.devices()[:NCORES]
    mesh = Mesh(np.asarray(devices), ("core",))
    in_specs = (PartitionSpec("core"),) * (n_params + n_outs)
    out_specs = (PartitionSpec("core"),) * len(out_names)
    sharded = jax.jit(
        shard_map(_body, mesh=mesh, in_specs=in_specs, out_specs=out_specs,
                  check_rep=False),
        donate_argnums=donate, keep_unused=True)

    per_core = [[np.asarray(m[name]) for name in in_names] for m in in_maps]
    concat_in = [np.concatenate([per_core[c][i] for c in range(NCORES)], axis=0)
                 for i in range(n_params)]
    shard = NamedSharding(mesh, PartitionSpec("core"))
    dev_in = [jax.device_put(a, shard) for a in concat_in]
    mk_zeros = lambda: [jax.device_put(
        np.zeros((NCORES * z.shape[0], *z.shape[1:]), z.dtype), shard)
        for z in zero_outs]

    # warmup (compile + first runs)
    for _ in range(3):
        outs = sharded(*dev_in, *mk_zeros())
        jax.block_until_ready(outs)

    all_zeros = [mk_zeros() for _ in range(iters)]
    t0 = time.perf_counter()
    all_outs = [sharded(*dev_in, *z) for z in all_zeros]
    jax.block_until_ready(all_outs)
    t1 = time.perf_counter()
    return (t1 - t0) / iters


def _make_in_maps(**inputs):
    import ml_dtypes
    g = np.asarray(inputs["gradients"], np.float32).reshape(B)
    p = np.asarray(inputs["parameters"], np.float32).reshape(B)
    mom = np.asarray(inputs["momentum"], np.float32).reshape(B)
    h0 = np.asarray(inputs["h0"], np.float32)
    c0 = np.asarray(inputs["c0"], np.float32)
    W1 = np.asarray(inputs["W1"], np.float32)
    b1 = np.asarray(inputs["b1"], np.float32)
    W2 = np.asarray(inputs["W2"], np.float32)
    b2 = np.asarray(inputs["b2"], np.float32)
    W_ih = np.asarray(inputs["W_ih"], np.float32)
    W_hh = np.asarray(inputs["W_hh"], np.float32)
    b_ih = np.asarray(inputs["b_ih"], np.float32)
    b_hh = np.asarray(inputs["b_hh"], np.float32)
    W_out = np.asarray(inputs["W_out"], np.float32)
    b_out = np.asarray(inputs["b_out"], np.float32)
    bf16 = ml_dtypes.bfloat16

    S1 = np.stack([W1[:, 0] + W1[:, 2], W1[:, 1], 0.9 * W1[:, 2]]).astype(np.float32)
    b1c = b1.reshape(H, 1).astype(np.float32)
    Wf = W_ih @ W2
    bg = b_ih + b_hh + W_ih @ b2
    iA = list(range(64, 128)) + list(range(0, 64))
    iB = list(range(192, 256)) + list(range(128, 192))

    def chunk(idx):
        return np.concatenate([Wf[idx, :].T, W_hh[idx, :].T], axis=0)

    SGA = chunk(iA).astype(bf16)
    SGB = chunk(iB).astype(bf16)
    biasA = (0.5 * bg[iA]).reshape(128, 1).astype(np.float32)
    biasB = np.concatenate([0.5 * bg[iB[:64]], bg[iB[64:]]]) \
        .reshape(128, 1).astype(np.float32)
    scaleB = np.concatenate([np.full(64, 0.5), np.full(64, 1.0)]) \
        .reshape(128, 1).astype(np.float32)
    Z = np.zeros((H, 127), np.float32)
    Z[:, 63] = 0.5 * W_out[0]
    Z = Z.astype(bf16)
    bout_v = np.full((H, 1), b_out[0], np.float32)

    feats3 = np.stack([g, p, mom])
    h0T = np.ascontiguousarray(h0.T).astype(bf16)
    c0T = np.ascontiguousarray(c0.T).astype(bf16)

    in_maps = []
    for c in range(NCORES):
        sl = slice(c * R, (c + 1) * R)
        in_maps.append({
            "feats3": np.ascontiguousarray(feats3[:, sl]),
            "h0T": np.ascontiguousarray(h0T[:, sl]),
            "c0T": np.ascontiguousarray(c0T[:, sl]),
            "S1": S1, "b1c": b1c, "SGA": SGA, "SGB": SGB, "Z": Z,
            "biasA": biasA, "biasB": biasB, "scaleB": scaleB,
            "b_out_v": bout_v,
        })
    return in_maps
